# revision 5
# baseline (speedup 1.0000x reference)
"""Trainium2 kernel for nn_AttentionConstrainedLoss.

Strategy (8 NeuronCores, full inputs in / full output out):
  - The memory-heavy part is the per-grid unbiased variance over D=128 of
    atten_map [B=4, HW=65536, D=128] fp32 (128 MiB).  Sharding: data-parallel
    over B (4 scenes) x sequence-sharded over HW (2 halves) = 8 cores, each
    streaming a 16 MiB chunk and emitting 32768 per-grid M2 values (the
    1/127 unbiased-variance scale is applied on host).
  - Per-cell reduction costs: DVE bn_stats 194ns, ACT Copy+Square w/ accum
    958ns; together they barely cover 256 cells/partition inside the ~47us
    DMA window, so every non-reduction op is moved OFF those engines: all
    section combines/finalizes run on the otherwise-idle Pool (GPSIMD)
    engine.  ~45 cells go to ACT (2 in early blocks, 1 mid, 2 in the last
    body blocks so the DVE stream runs light into the endgame).
  - Input streams as 31 blocks of 8 cells/partition + the last 8 cells as
    eight 1-cell pieces: each 182ns piece's bn_stats (194ns) trails its
    DMA-sem by only ~20ns, so the last cell's stats complete ~1.25us after
    the final byte lands (the 900ns DMA-sem propagation is the floor).
  - The final 8-cell combine runs on DVE right after the last bn_stats;
    a single SP dma_start stores the whole vtile, so exactly one store
    (HWDGE 625 + DGE 650 + 182 + 900ns sem) sits in the kernel tail.
  - The box -> grid assignment (point-in-rotated-rect over a 0.4 m grid,
    sequential overlap-kill scan, segment means) touches only ~400 cells per
    box (boxes are <= 5 m).  It is exact, tiny, and done on host in fp32
    numpy replicating the reference semantics including scan order and
    argmin tie-breaking.
"""

import numpy as np

# ---------------------------------------------------------------------------
# Problem constants (hardcoded per contract; kernel.py must be self-contained)
# ---------------------------------------------------------------------------
B, M, D = 4, 100, 128
H, W = 256, 256
HW = H * W
N_CORES = 8
HALF = HW // 2  # grid rows per core (sequence shard)
P = 128  # SBUF partitions
TPP = HALF // P  # grid cells per partition per core (256)
NBLK = 31  # full 8-cell body blocks; cells 248..255 stream as 1-cell pieces
CPB = 8

_PC_RANGE = np.asarray([-51.2, -51.2, -5.0, 51.2, 51.2, 3.0], dtype=np.float32)
_DIMS = _PC_RANGE[3:] - _PC_RANGE[:3]
_EFF_MIN, _EFF_MAX = np.float32(1.0), np.float32(6.0)

_NC_CACHE = {}
_CFG = {}


def _act_plan():
    """ACT cells per body block (trailing positions of each block)."""
    plan = _CFG.get("act_plan")
    if plan is None:
        plan = [2] * 10 + [1] * 17 + [2] * 4
    assert len(plan) == NBLK
    return plan


def _build_bass_program():
    """Per-core program: atten chunk [32768, 128] f32 -> M2 [128, 256] f32.

    Partition layout: grid cell g_local = p * 256 + t  (p = partition,
    t = free index).  Each partition reads 256*128 = 32768 contiguous fp32
    from HBM, so DMA descriptors stay fully contiguous per partition.
    """
    import concourse.bacc as bacc
    import concourse.mybir as mybir
    from concourse import tile

    f32 = mybir.dt.float32
    mult, add = mybir.AluOpType.mult, mybir.AluOpType.add

    nc = bacc.Bacc("TRN2", target_bir_lowering=False, debug=False)
    atten = nc.dram_tensor("atten", [HALF, D], f32, kind="ExternalInput")
    v_out = nc.dram_tensor("v_out", [P, TPP], f32, kind="ExternalOutput")

    # [128, 32768] view: partition p <- rows [p*256, (p+1)*256), contiguous.
    av = atten[:, :].rearrange("(p t) d -> p (t d)", p=P)

    act_plan = _act_plan()
    n_act = sum(act_plan)
    # section boundaries in BODY BLOCKS for the Pool-side combine/finalize;
    # the split cells [248,256) form their own DVE-side final combine
    sec_blocks = _CFG.get("sec_blocks", [0, 16, NBLK])
    comb_engine = _CFG.get("comb_engine", "gpsimd")  # mid combines/finalizes

    with tile.TileContext(nc) as tc:
        with (
            tc.tile_pool(name="io", bufs=int(_CFG.get("bufs", 10))) as io_pool,
            tc.tile_pool(name="scr", bufs=2) as scr_pool,
            tc.tile_pool(name="acc", bufs=1) as acc_pool,
            tc.tile_pool(name="tmp", bufs=2) as tmp_pool,
        ):
            stats = acc_pool.tile([P, TPP * 6], f32, tag="stats")
            st4 = stats[:].rearrange("p (b c u) -> p b c u", c=CPB, u=6)
            vtile = acc_pool.tile([P, TPP], f32, tag="vtile")
            vt3 = vtile[:].rearrange("p (b c) -> p b c", c=CPB)
            sums = acc_pool.tile([P, max(n_act, 1)], f32, tag="sums")
            sumsq = acc_pool.tile([P, max(n_act, 1)], f32, tag="sumsq")

            eng = getattr(nc, comb_engine)

            def combine_run(b0, b1, nd):
                """vt3[:, b0:b1, 0:nd] = cv_e+cv_o+32*(m_e-m_o)^2 (M2)."""
                nb = b1 - b0
                m_e = st4[:, b0:b1, 0:nd, 1]
                cv_e = st4[:, b0:b1, 0:nd, 2]
                m_o = st4[:, b0:b1, 0:nd, 4]
                cv_o = st4[:, b0:b1, 0:nd, 5]
                t_d = tmp_pool.tile([P, nb * nd], f32, tag="t_d")
                t_c = tmp_pool.tile([P, nb * nd], f32, tag="t_c")
                t_d3 = t_d[:].rearrange("p (s c) -> p s c", c=nd)
                t_c3 = t_c[:].rearrange("p (s c) -> p s c", c=nd)
                eng.tensor_sub(out=t_d3, in0=m_e, in1=m_o)
                eng.tensor_tensor(out=t_d[:], in0=t_d[:], in1=t_d[:], op=mult)
                eng.tensor_add(out=t_c3, in0=cv_e, in1=cv_o)
                eng.scalar_tensor_tensor(
                    out=vt3[:, b0:b1, 0:nd],
                    in0=t_d3,
                    scalar=32.0,
                    in1=t_c3,
                    op0=mult,
                    op1=add,
                )

            def act_runs(b0, b1):
                """Uniform-act-count runs [(blk0, blk1, na)] within blocks."""
                runs = []
                for b in range(b0, b1):
                    na = act_plan[b]
                    if runs and runs[-1][2] == na and runs[-1][1] == b:
                        runs[-1][1] = b + 1
                    else:
                        runs.append([b, b + 1, na])
                return [tuple(r) for r in runs]

            def finalize_sec(b0, b1, a0):
                """ACT cells of blocks [b0,b1): M2 = sumsq - sum^2/128."""
                na_s = sum(act_plan[b] for b in range(b0, b1))
                if na_s == 0:
                    return
                t_u = tmp_pool.tile([P, na_s], f32, tag="t_u")
                eng.tensor_tensor(
                    out=t_u[:],
                    in0=sums[:, a0 : a0 + na_s],
                    in1=sums[:, a0 : a0 + na_s],
                    op=mult,
                )
                eng.scalar_tensor_tensor(
                    out=t_u[:],
                    in0=t_u[:],
                    scalar=float(-1.0 / 128.0),
                    in1=sumsq[:, a0 : a0 + na_s],
                    op0=mult,
                    op1=add,
                )
                off = 0
                for rb0, rb1, na in act_runs(b0, b1):
                    if na == 0:
                        continue
                    nb = rb1 - rb0
                    eng.tensor_scalar_mul(
                        vt3[:, rb0:rb1, CPB - na : CPB],
                        t_u[:, off : off + nb * na].rearrange(
                            "p (s c) -> p s c", c=na
                        ),
                        1.0,
                    )
                    off += nb * na

            def emit_sec_exact(si):
                b0, b1 = sec_blocks[si], sec_blocks[si + 1]
                a0 = sum(act_plan[b] for b in range(0, b0))
                base_nd = min(CPB - act_plan[b] for b in range(b0, b1))
                combine_run(b0, b1, base_nd)
                for rb0, rb1, na in act_runs(b0, b1):
                    nd = CPB - na
                    if nd > base_nd:
                        combine_cols(rb0, rb1, base_nd, nd)
                finalize_sec(b0, b1, a0)

            def combine_cols(b0, b1, c0, c1):
                nb, ncol = b1 - b0, c1 - c0
                m_e = st4[:, b0:b1, c0:c1, 1]
                cv_e = st4[:, b0:b1, c0:c1, 2]
                m_o = st4[:, b0:b1, c0:c1, 4]
                cv_o = st4[:, b0:b1, c0:c1, 5]
                t_d = tmp_pool.tile([P, nb * ncol], f32, tag="t_d")
                t_c = tmp_pool.tile([P, nb * ncol], f32, tag="t_c")
                t_d3 = t_d[:].rearrange("p (s c) -> p s c", c=ncol)
                t_c3 = t_c[:].rearrange("p (s c) -> p s c", c=ncol)
                eng.tensor_sub(out=t_d3, in0=m_e, in1=m_o)
                eng.tensor_tensor(out=t_d[:], in0=t_d[:], in1=t_d[:], op=mult)
                eng.tensor_add(out=t_c3, in0=cv_e, in1=cv_o)
                eng.scalar_tensor_tensor(
                    out=vt3[:, b0:b1, c0:c1],
                    in0=t_d3,
                    scalar=32.0,
                    in1=t_c3,
                    op0=mult,
                    op1=add,
                )

            sec_i = 0
            a_idx = 0
            for b in range(NBLK):
                slab = io_pool.tile([P, CPB * D], f32, tag="slab")
                nc.sync.dma_start(
                    out=slab[:], in_=av[:, b * CPB * D : (b + 1) * CPB * D]
                )
                na = act_plan[b]
                nd = CPB - na
                for k in range(nd):
                    nc.vector.bn_stats(
                        out=st4[:, b, k, :],
                        in_=slab[:, k * D : (k + 1) * D],
                    )
                for j in range(na):
                    chunk = slab[:, (nd + j) * D : (nd + j + 1) * D]
                    s1 = scr_pool.tile([P, D], f32, tag="scr")
                    nc.scalar.activation(
                        out=s1[:],
                        in_=chunk,
                        func=mybir.ActivationFunctionType.Copy,
                        accum_out=sums[:, a_idx : a_idx + 1],
                    )
                    s2 = scr_pool.tile([P, D], f32, tag="scr")
                    nc.scalar.activation(
                        out=s2[:],
                        in_=chunk,
                        func=mybir.ActivationFunctionType.Square,
                        accum_out=sumsq[:, a_idx : a_idx + 1],
                    )
                    a_idx += 1
                while sec_i < len(sec_blocks) - 1 and b + 1 >= sec_blocks[sec_i + 1]:
                    emit_sec_exact(sec_i)
                    sec_i += 1

            # tail: cells [248,256) as 1-cell pieces, all DVE
            for k in range(CPB):
                cell = NBLK * CPB + k
                pk = io_pool.tile([P, D], f32, tag="pk")
                nc.sync.dma_start(
                    out=pk[:], in_=av[:, cell * D : (cell + 1) * D]
                )
                nc.vector.bn_stats(out=st4[:, NBLK, k, :], in_=pk[:])

            # final combine on DVE (it is idle right after the last bn_stats)
            m_e = st4[:, NBLK, :, 1]
            cv_e = st4[:, NBLK, :, 2]
            m_o = st4[:, NBLK, :, 4]
            cv_o = st4[:, NBLK, :, 5]
            t_d = tmp_pool.tile([P, CPB], f32, tag="t_dz")
            t_c = tmp_pool.tile([P, CPB], f32, tag="t_cz")
            nc.vector.tensor_sub(out=t_d[:], in0=m_e, in1=m_o)
            nc.vector.tensor_tensor(out=t_d[:], in0=t_d[:], in1=t_d[:], op=mult)
            nc.vector.tensor_add(out=t_c[:], in0=cv_e, in1=cv_o)
            nc.vector.scalar_tensor_tensor(
                out=vt3[:, NBLK, :],
                in0=t_d[:],
                scalar=32.0,
                in1=t_c[:],
                op0=mult,
                op1=add,
            )

            # single store of the full result
            nc.sync.dma_start(out=v_out[:, :], in_=vtile[:])

    nc.compile()
    return nc


def _get_nc():
    if "nc" not in _NC_CACHE:
        _NC_CACHE["nc"] = _build_bass_program()
    return _NC_CACHE["nc"]


def _device_variance(atten_map: np.ndarray, trace: bool = False):
    """Run the SPMD kernel on 8 cores. Returns per-grid M2 [B, HW] f32
    (unbiased variance times 127; scaled on host)."""
    from concourse.bass_utils import run_bass_kernel_spmd

    nc = _get_nc()
    in_maps = []
    for c in range(N_CORES):
        b, h = c // 2, c % 2
        # slice BEFORE materializing so jax-array inputs transfer in 16 MiB
        # per-core pieces (large single device->host copies can fail)
        chunk = atten_map[b, h * HALF : (h + 1) * HALF, :]
        chunk = np.ascontiguousarray(np.asarray(chunk), dtype=np.float32)
        in_maps.append({"atten": chunk})
    res = run_bass_kernel_spmd(nc, in_maps, list(range(N_CORES)), trace=trace)
    v = np.empty((B, HW), dtype=np.float32)
    for c in range(N_CORES):
        b, h = c // 2, c % 2
        v[b, h * HALF : (h + 1) * HALF] = res.results[c]["v_out"].reshape(HALF)
    return v, res


# ---------------------------------------------------------------------------
# Host-side box logic (exact fp32 replication of the reference semantics)
# ---------------------------------------------------------------------------
def _grid_axis_vals():
    gx = (np.arange(W, dtype=np.float32) + np.float32(0.5)) / np.float32(W) * _DIMS[
        0
    ] + _PC_RANGE[0]
    gy = (np.arange(H, dtype=np.float32) + np.float32(0.5)) / np.float32(H) * _DIMS[
        1
    ] + _PC_RANGE[1]
    return gx, gy


_CORNERS_NORM = np.asarray(
    [[-0.5, -0.5], [-0.5, 0.5], [0.5, 0.5], [0.5, -0.5]], dtype=np.float32
)


def _scene_loss(v: np.ndarray, boxes: np.ndarray, gx: np.ndarray, gy: np.ndarray):
    centers = boxes[:, :2]
    lw = boxes[:, 3:5]
    angles = boxes[:, 6]
    ratio_l = np.clip(_DIMS[0] / np.float32(W) / lw[:, 0], _EFF_MIN, _EFF_MAX)
    ratio_w = np.clip(_DIMS[1] / np.float32(H) / lw[:, 1], _EFF_MIN, _EFF_MAX)
    eff = np.stack([lw[:, 0] * ratio_l, lw[:, 1] * ratio_w], axis=1)
    corners = eff[:, None, :] * _CORNERS_NORM  # [M, 4, 2]
    c = np.cos(angles)[:, None]
    s = np.sin(angles)[:, None]
    rx = corners[..., 0] * c + corners[..., 1] * s
    ry = -corners[..., 0] * s + corners[..., 1] * c
    corners = np.stack([rx, ry], axis=-1) + centers[:, None, :]  # [M, 4, 2]
    edges = np.roll(corners, -1, axis=1) - corners

    # exact argmin (first-index tie-break) of d2 over the full grid, as in ref
    d2 = (gx[None, None, :] - centers[:, 0:1, None]) ** 2 + (
        gy[None, :, None] - centers[:, 1:2, None]
    ) ** 2  # [M, H, W] f32
    nearest_g = np.argmin(d2.reshape(M, HW), axis=1)

    flag = np.full(HW, -1, dtype=np.int32)
    for i in range(M):
        cmin, cmax = corners[i, :, 0].min(), corners[i, :, 0].max()
        rmin, rmax = corners[i, :, 1].min(), corners[i, :, 1].max()
        c0 = max(0, int(np.searchsorted(gx, cmin)) - 1)
        c1 = min(W, int(np.searchsorted(gx, cmax)) + 1)
        r0 = max(0, int(np.searchsorted(gy, rmin)) - 1)
        r1 = min(H, int(np.searchsorted(gy, rmax)) + 1)
        dx = gx[None, None, c0:c1] - corners[i, :, 0][:, None, None]
        dy = gy[None, r0:r1, None] - corners[i, :, 1][:, None, None]
        cross = (
            edges[i, :, 0][:, None, None] * dy - edges[i, :, 1][:, None, None] * dx
        )
        inside = np.all(cross >= 0, axis=0) | np.all(cross <= 0, axis=0)
        rr, cc = np.nonzero(inside)
        gidx = (rr + r0).astype(np.int64) * W + (cc + c0)
        gidx = np.union1d(gidx, np.asarray([nearest_g[i]]))
        cur = flag[gidx]
        flag[gidx] = np.where(cur == -1, np.int32(i), np.int32(-1))

    sums = np.zeros(M, dtype=np.float32)
    cnts = np.zeros(M, dtype=np.float32)
    msk = flag >= 0
    np.add.at(sums, flag[msk], v[msk])
    np.add.at(cnts, flag[msk], np.float32(1.0))
    sums *= np.float32(1.0 / 127.0)  # device emits M2; unbiased var = M2/127
    valid = cnts > 0
    box_mean = sums / np.maximum(cnts, np.float32(1.0))
    loss = -np.sum(box_mean[valid], dtype=np.float32)
    return loss, np.float32(np.sum(valid))


def _host_reduce(v: np.ndarray, gt_bboxes: np.ndarray):
    gx, gy = _grid_axis_vals()
    losses = np.zeros(B, dtype=np.float32)
    nums = np.zeros(B, dtype=np.float32)
    for b in range(B):
        losses[b], nums[b] = _scene_loss(
            v[b], np.asarray(gt_bboxes[b], dtype=np.float32), gx, gy
        )
    var_loss = np.sum(losses, dtype=np.float32)
    var_pos_num = np.maximum(np.sum(nums, dtype=np.float32), np.float32(1.0))
    return np.asarray(np.float32(var_loss / var_pos_num))


def kernel(atten_map: np.ndarray, gt_bboxes: np.ndarray, gt_labels: np.ndarray):
    gt_bboxes = np.asarray(gt_bboxes, dtype=np.float32)
    v, _ = _device_variance(atten_map)
    return _host_reduce(v, gt_bboxes)
